# revision 25
# baseline (speedup 1.0000x reference)
"""Self-contained Trainium2 Bass kernel: per-channel 3x3-window attention
(nn_AttentionConv).  Runs SPMD on 8 NeuronCores, data-parallel over batch
(B=8 -> one batch element per core, no collectives).

Math per (b, c, h, w):
  q = wq @ y;  k = wk @ pad(x);  v = wv @ pad(x)          (1x1 convs)
  logit[t] = q * (k_win[t] + rel[t]),  t over the 3x3 window
    rel[t] = rel_h[c, di] for c < 128, rel_w[c-128, dj] otherwise
  out = sum_t softmax_t(logit) * v_win[t]

Engine plan (per core):
  TensorE : f32r QKV matmuls; identity-matmul PSUM accumulation of the
            softmax numerator/denominator (absorbs all elementwise adds)
  VectorE : fused (k_shift + rel)*q logits (fp32, scalar_tensor_tensor),
            bf16 e*v_shift products at 2x, final numer*recip(denom)
  ScalarE : exp (fp32 in -> bf16 out), PSUM evacuations
  Softmax runs without max-subtraction: |logit| < ~45 for this input
  scale, exp stays inside fp32/bf16 range.
"""

import json
from contextlib import ExitStack

import numpy as np

import concourse.bass as bass
import concourse.tile as tile
from concourse import mybir
from concourse.masks import make_identity

# ---------------------------------------------------------------- constants
P = 128          # SBUF partitions
C = 256          # channels in/out
H = W = 64
HP = WP = 66     # padded spatial
RCHUNK = 16      # rows per attention chunk (psum: 2 banks per accumulator)
POS = [(di, dj) for di in range(3) for dj in range(3)]
FP32 = mybir.dt.float32
F32R = mybir.dt.float32r
BF16 = mybir.dt.bfloat16
N_CORES = 8

# --------------------------------------------------------------- BIR fixup
# This container's walrus build accepts at most ONE sync wait per
# instruction; Tile can emit more.  Split extras onto same-engine NoOps
# inserted immediately before the instruction.


def _fix_bir_waits(bir_json: bytes) -> bytes:
    j = json.loads(bir_json)
    n = 0
    for f in j.get("functions", []):
        for b in f.get("blocks", []):
            out = []
            for inst in b.get("instructions", []):
                si = inst.get("sync_info")
                waits = (si or {}).get("on_wait") or []
                if len(waits) > 1:
                    for w in waits[:-1]:
                        n += 1
                        out.append({
                            "debug": inst.get("debug", 0),
                            "engine": inst["engine"],
                            "ins": [],
                            "outs": [],
                            "name": f"WFIX-{n}",
                            "opcode": "NoOp",
                            "sync_info": {"on_update": [], "on_wait": [w]},
                        })
                    si["on_wait"] = [waits[-1]]
                out.append(inst)
            b["instructions"] = out
    return json.dumps(j).encode()


_PATCHED = False


def _patch_compiler():
    global _PATCHED
    if _PATCHED:
        return
    import concourse.bass2jax as bass2jax
    import concourse.bass_utils as bass_utils

    orig = bass_utils.compile_bir_kernel

    def patched(bir_json, tmpdir, neff_name="file.neff"):
        if isinstance(bir_json, str):
            bir_json = bir_json.encode()
        return orig(_fix_bir_waits(bir_json), tmpdir, neff_name)

    bass_utils.compile_bir_kernel = patched
    bass2jax.compile_bir_kernel = patched
    _PATCHED = True


def _T(pool, shape, dtype, nm):
    return pool.tile(shape, dtype, name=nm, tag=nm)


# ------------------------------------------------------------ kernel build
def build_nc(reps: int = 1) -> bass.Bass:
    nc = bass.Bass()
    x = nc.declare_dram_parameter("x", [C, H, W], FP32, isOutput=False)
    y = nc.declare_dram_parameter("y", [C, H, W], FP32, isOutput=False)
    wq = nc.declare_dram_parameter("wq", [C, C], FP32, isOutput=False)
    wk = nc.declare_dram_parameter("wk", [C, C], FP32, isOutput=False)
    wv = nc.declare_dram_parameter("wv", [C, C], FP32, isOutput=False)
    relh = nc.declare_dram_parameter("relh", [P, 3], FP32, isOutput=False)
    relw = nc.declare_dram_parameter("relw", [P, 3], FP32, isOutput=False)
    out = nc.declare_dram_parameter("out", [C, H, W], FP32, isOutput=True)

    ADD = mybir.AluOpType.add
    MULT = mybir.AluOpType.mult
    EXP = mybir.ActivationFunctionType.Exp

    with tile.TileContext(nc) as tc, ExitStack() as ctx:
        consts = ctx.enter_context(tc.tile_pool(name="consts", bufs=1))
        inpool = ctx.enter_context(tc.tile_pool(name="inpool", bufs=1))
        ldp = ctx.enter_context(tc.tile_pool(name="ldp", bufs=4))
        wpool = ctx.enter_context(tc.tile_pool(name="wpool", bufs=1))
        big = ctx.enter_context(tc.tile_pool(name="big", bufs=1))
        lwork = ctx.enter_context(tc.tile_pool(name="lwork", bufs=2))
        ework = ctx.enter_context(tc.tile_pool(name="ework", bufs=3))
        uwork = ctx.enter_context(tc.tile_pool(name="uwork", bufs=4))
        fwork = ctx.enter_context(tc.tile_pool(name="fwork", bufs=1))
        outp = ctx.enter_context(tc.tile_pool(name="outp", bufs=2))
        qkv_ps = ctx.enter_context(tc.tile_pool(name="qkv_ps", bufs=2, space="PSUM"))
        acc_ps = ctx.enter_context(tc.tile_pool(name="acc_ps", bufs=1, space="PSUM"))

        ident = _T(consts, [P, P], BF16, "ident")
        make_identity(nc, ident)
        ident_f = _T(consts, [P, P], FP32, "ident_f")
        make_identity(nc, ident_f)
        relh_sb = _T(consts, [P, 3], FP32, "relh")
        nc.scalar.dma_start(out=relh_sb, in_=relh[:, :])
        relw_sb = _T(consts, [P, 3], FP32, "relw")
        nc.scalar.dma_start(out=relw_sb, in_=relw[:, :])

        # ---- weights: wT[name][:, cit, cot*128:...] = w[cot-blk, cit-blk]^T
        wT = {}
        for name, wdram in (("q", wq), ("k", wk), ("v", wv)):
            wT[name] = _T(wpool, [P, 2, C], F32R, f"wT_{name}")
            for cot in range(2):
                wrow = _T(wpool, [P, C], FP32, "wrow")
                nc.scalar.dma_start(out=wrow, in_=wdram[cot * P:(cot + 1) * P, :])
                for cit in range(2):
                    ps = _T(qkv_ps, [P, P], FP32, "qkv_ps_t")
                    nc.tensor.transpose(
                        ps, in_=wrow[:, cit * P:(cit + 1) * P], identity=ident_f)
                    nc.scalar.copy(
                        out=wT[name][:, cit, cot * P:(cot + 1) * P], in_=ps)

        # (reps>1 repeats the whole load+compute for hardware timing)
        for _rep in range(reps):
            _build_body(nc, x, y, relh_sb, relw_sb, wT, ident,
                        inpool, ldp, big, lwork, ework, uwork, fwork, outp,
                        qkv_ps, acc_ps, out)
    return nc


def _build_body(nc, x, y, relh_sb, relw_sb, wT, ident,
                inpool, ldp, big, lwork, ework, uwork, fwork, outp,
                qkv_ps, acc_ps, out):
        ADD = mybir.AluOpType.add
        MULT = mybir.AluOpType.mult
        EXP = mybir.ActivationFunctionType.Exp

        # ---- inputs: DMA fp32 chunks, round to f32r (matmul operand dtype).
        # x first (k/v matmuls gate the attention pipeline), chunks
        # alternating between the two HWDGE queues (SP / Activation).
        x_r = [_T(inpool, [P, H, W], F32R, f"xr{cit}") for cit in range(2)]
        y_r = [_T(inpool, [P, H, W], F32R, f"yr{cit}") for cit in range(2)]
        qi = 0
        for dram, dsts in ((x, x_r), (y, y_r)):
            for r0 in range(0, H, 16):
                for cit in range(2):
                    sc = _T(ldp, [P, 16, W], FP32, "ld_scratch")
                    eng = nc.sync if qi % 2 == 0 else nc.scalar
                    eng.dma_start(
                        out=sc, in_=dram[cit * P:(cit + 1) * P, r0:r0 + 16, :])
                    # rounding copies: first band on the (startup-idle) DVE
                    # so band-0 matmuls start ASAP, the rest on ScalarE
                    qi += 1
                    if r0 == 0:
                        nc.vector.tensor_copy(
                            out=dsts[cit][:, r0:r0 + 16, :], in_=sc)
                    else:
                        nc.scalar.copy(out=dsts[cit][:, r0:r0 + 16, :], in_=sc)

        for cot in range(2):
            rel_sb = relh_sb if cot == 0 else relw_sb

            q_sb = _T(big, [P, H, W], FP32, "q_sb")
            kpad = _T(big, [P, HP, WP], FP32, "kpad")
            vpad = _T(big, [P, HP, WP], BF16, "vpad")
            vpad_o = _T(big, [P, HP, WP], BF16, "vpad_o")
            # zero only the padding borders (interior is fully overwritten
            # by the QKV evacuations): top+bottom rows, then left+right cols
            for t in (kpad, vpad, vpad_o):
                nc.gpsimd.memset(t[:, 0:HP:HP - 1, :], 0.0)
                nc.gpsimd.memset(t[:, 1:HP - 1, 0:WP:WP - 1], 0.0)

            # ---- QKV 1x1 convs (f32r matmuls, contraction over Cin).
            # 16-row psum tiles (2 banks); each matmul targets one bank,
            # evacuations amortize the ScalarE per-instruction overhead.
            for b in range(4):  # bands of 16 rows = 1024 sites
                r = b * 16
                for wname in ("q", "k", "v"):
                    src = y_r if wname == "q" else x_r
                    ps = _T(qkv_ps, [P, 16, W], FP32, "qkv_ps_t")
                    for hb in range(2):
                        for cit in range(2):
                            nc.tensor.matmul(
                                ps[:, hb * 8:hb * 8 + 8, :],
                                lhsT=wT[wname][:, cit, cot * P:(cot + 1) * P],
                                rhs=src[cit][:, r + hb * 8:r + hb * 8 + 8, :],
                                start=(cit == 0),
                                stop=(cit == 1),
                            )
                    if wname == "q":
                        # q stays fp32 (logit path precision)
                        nc.scalar.copy(out=q_sb[:, r:r + 16, :], in_=ps)
                    elif wname == "k":
                        nc.scalar.copy(
                            out=kpad[:, 1 + r:17 + r, 1:1 + W], in_=ps)
                    else:
                        # v in bf16, plus a one-element-left-shifted copy so
                        # dj=1 window reads stay 4B-aligned (DVE 2x mode)
                        nc.scalar.copy(
                            out=vpad[:, 1 + r:17 + r, 1:1 + W], in_=ps)
                        nc.scalar.copy(
                            out=vpad_o[:, 1 + r:17 + r, 0:W], in_=ps)

            # ---- attention, chunks of RCHUNK rows
            # positions grouped 3-at-a-time sharing one rel scalar, so exp
            # runs as one ScalarE pass per group:
            #   cot0: rel depends on di -> group g = di, member m = dj
            #   cot1: rel depends on dj -> group g = dj, member m = di
            for r0 in range(0, H, RCHUNK):
                denom_ps = _T(acc_ps, [P, RCHUNK, W], FP32, "denom_ps")
                numer_ps = _T(acc_ps, [P, RCHUNK, W], FP32, "numer_ps")
                qv = q_sb[:, r0:r0 + RCHUNK, :]
                for g in range(3):
                    scal = rel_sb[:, g:g + 1]
                    l3 = _T(lwork, [P, 3, RCHUNK, W], FP32, "l3")
                    for m in range(3):
                        di, dj = (g, m) if cot == 0 else (m, g)
                        ksh = kpad[:, r0 + di:r0 + di + RCHUNK, dj:dj + W]
                        nc.vector.scalar_tensor_tensor(
                            out=l3[:, m, :, :], in0=ksh, scalar=scal, in1=qv,
                            op0=ADD, op1=MULT)
                    e3 = _T(ework, [P, 3, RCHUNK, W], BF16, "e3")
                    nc.scalar.activation(out=e3, in_=l3, func=EXP)
                    for m in range(3):
                        di, dj = (g, m) if cot == 0 else (m, g)
                        if dj == 1:
                            vsh = vpad_o[:, r0 + di:r0 + di + RCHUNK, 0:W]
                        else:
                            vsh = vpad[:, r0 + di:r0 + di + RCHUNK, dj:dj + W]
                        e_t = e3[:, m, :, :]
                        u_t = _T(uwork, [P, RCHUNK, W], BF16, "u_t")
                        nc.vector.tensor_mul(out=u_t, in0=e_t, in1=vsh)

                        first, last = (g == 0 and m == 0), (g == 2 and m == 2)
                        for hb in range(2):  # one matmul per psum bank
                            sl = slice(hb * 8, hb * 8 + 8)
                            nc.tensor.matmul(
                                denom_ps[:, sl, :], lhsT=ident, rhs=e_t[:, sl, :],
                                start=first, stop=last)
                            nc.tensor.matmul(
                                numer_ps[:, sl, :], lhsT=ident, rhs=u_t[:, sl, :],
                                start=first, stop=last)

                # 1/denom as exp(-ln(denom)) on ScalarE (same activation
                # table set as Exp; custom-DVE reciprocal doesn't compile
                # on this toolchain)
                lnd = _T(fwork, [P, RCHUNK, W], FP32, "lnd")
                nc.scalar.activation(out=lnd, in_=denom_ps,
                                     func=mybir.ActivationFunctionType.Ln)
                rec = _T(fwork, [P, RCHUNK, W], FP32, "rec")
                nc.scalar.activation(out=rec, in_=lnd, func=EXP, scale=-1.0)
                o_t = _T(outp, [P, RCHUNK, W], FP32, "o_t")
                nc.vector.tensor_mul(out=o_t, in0=numer_ps, in1=rec)
                nc.sync.dma_start(
                    out=out[cot * P:(cot + 1) * P, r0:r0 + RCHUNK, :], in_=o_t)


# ------------------------------------------------------------ entry points
def make_in_maps(x, y, wq, wk, wv, rel_h, rel_w):
    relh = np.ascontiguousarray(rel_h[:, 0, 0, :, 0], dtype=np.float32)  # [128,3]
    relw = np.ascontiguousarray(rel_w[:, 0, 0, 0, :], dtype=np.float32)  # [128,3]
    shared = {
        "wq": np.ascontiguousarray(wq, np.float32),
        "wk": np.ascontiguousarray(wk, np.float32),
        "wv": np.ascontiguousarray(wv, np.float32),
        "relh": relh,
        "relw": relw,
    }
    maps = []
    for i in range(N_CORES):
        maps.append({
            "x": np.ascontiguousarray(x[i], np.float32),
            "y": np.ascontiguousarray(y[i], np.float32),
            **shared,
        })
    return maps


_CACHED_NC = None


def kernel(x, y, wq, wk, wv, rel_h, rel_w):
    global _CACHED_NC
    _patch_compiler()
    from concourse.bass_utils import run_bass_kernel_spmd

    if _CACHED_NC is None:
        _CACHED_NC = build_nc()
    nc = _CACHED_NC
    in_maps = make_in_maps(x, y, wq, wk, wv, rel_h, rel_w)
    res = run_bass_kernel_spmd(nc, in_maps, core_ids=list(range(N_CORES)))
    out = np.stack([res.results[i]["out"] for i in range(N_CORES)], axis=0)
    return out.astype(np.float32)


# revision 26
# speedup vs baseline: 1.0414x; 1.0414x over previous
"""Self-contained Trainium2 Bass kernel: per-channel 3x3-window attention
(nn_AttentionConv).  Runs SPMD on 8 NeuronCores, data-parallel over batch
(B=8 -> one batch element per core, no collectives).

Math per (b, c, h, w):
  q = wq @ y;  k = wk @ pad(x);  v = wv @ pad(x)          (1x1 convs)
  logit[t] = q * (k_win[t] + rel[t]),  t over the 3x3 window
    rel[t] = rel_h[c, di] for c < 128, rel_w[c-128, dj] otherwise
  out = sum_t softmax_t(logit) * v_win[t]

Engine plan (per core):
  TensorE : f32r QKV matmuls; identity-matmul PSUM accumulation of the
            softmax numerator/denominator (absorbs all elementwise adds)
  VectorE : fused (k_shift + rel)*q logits (fp32, scalar_tensor_tensor),
            bf16 e*v_shift products at 2x, final numer*recip(denom)
  ScalarE : exp (fp32 in -> bf16 out), PSUM evacuations
  Softmax runs without max-subtraction: |logit| < ~45 for this input
  scale, exp stays inside fp32/bf16 range.
"""

import json
from contextlib import ExitStack

import numpy as np

import concourse.bass as bass
import concourse.tile as tile
from concourse import mybir
from concourse.masks import make_identity

# ---------------------------------------------------------------- constants
P = 128          # SBUF partitions
C = 256          # channels in/out
H = W = 64
HP = WP = 66     # padded spatial
RCHUNK = 16      # rows per attention chunk (psum: 2 banks per accumulator)
POS = [(di, dj) for di in range(3) for dj in range(3)]
FP32 = mybir.dt.float32
F32R = mybir.dt.float32r
BF16 = mybir.dt.bfloat16
N_CORES = 8

# --------------------------------------------------------------- BIR fixup
# This container's walrus build accepts at most ONE sync wait per
# instruction; Tile can emit more.  Split extras onto same-engine NoOps
# inserted immediately before the instruction.


def _fix_bir_waits(bir_json: bytes) -> bytes:
    j = json.loads(bir_json)
    n = 0
    for f in j.get("functions", []):
        for b in f.get("blocks", []):
            out = []
            for inst in b.get("instructions", []):
                si = inst.get("sync_info")
                waits = (si or {}).get("on_wait") or []
                if len(waits) > 1:
                    for w in waits[:-1]:
                        n += 1
                        out.append({
                            "debug": inst.get("debug", 0),
                            "engine": inst["engine"],
                            "ins": [],
                            "outs": [],
                            "name": f"WFIX-{n}",
                            "opcode": "NoOp",
                            "sync_info": {"on_update": [], "on_wait": [w]},
                        })
                    si["on_wait"] = [waits[-1]]
                out.append(inst)
            b["instructions"] = out
    return json.dumps(j).encode()


_PATCHED = False


def _patch_compiler():
    global _PATCHED
    if _PATCHED:
        return
    import concourse.bass2jax as bass2jax
    import concourse.bass_utils as bass_utils

    orig = bass_utils.compile_bir_kernel

    def patched(bir_json, tmpdir, neff_name="file.neff"):
        if isinstance(bir_json, str):
            bir_json = bir_json.encode()
        return orig(_fix_bir_waits(bir_json), tmpdir, neff_name)

    bass_utils.compile_bir_kernel = patched
    bass2jax.compile_bir_kernel = patched
    _PATCHED = True


def _T(pool, shape, dtype, nm):
    return pool.tile(shape, dtype, name=nm, tag=nm)


# ------------------------------------------------------------ kernel build
def build_nc(reps: int = 1) -> bass.Bass:
    nc = bass.Bass()
    x = nc.declare_dram_parameter("x", [C, H, W], FP32, isOutput=False)
    y = nc.declare_dram_parameter("y", [C, H, W], FP32, isOutput=False)
    wq = nc.declare_dram_parameter("wq", [C, C], FP32, isOutput=False)
    wk = nc.declare_dram_parameter("wk", [C, C], FP32, isOutput=False)
    wv = nc.declare_dram_parameter("wv", [C, C], FP32, isOutput=False)
    relh = nc.declare_dram_parameter("relh", [P, 3], FP32, isOutput=False)
    relw = nc.declare_dram_parameter("relw", [P, 3], FP32, isOutput=False)
    out = nc.declare_dram_parameter("out", [C, H, W], FP32, isOutput=True)

    ADD = mybir.AluOpType.add
    MULT = mybir.AluOpType.mult
    EXP = mybir.ActivationFunctionType.Exp

    with tile.TileContext(nc) as tc, ExitStack() as ctx:
        consts = ctx.enter_context(tc.tile_pool(name="consts", bufs=1))
        inpool = ctx.enter_context(tc.tile_pool(name="inpool", bufs=1))
        ldp = ctx.enter_context(tc.tile_pool(name="ldp", bufs=4))
        wpool = ctx.enter_context(tc.tile_pool(name="wpool", bufs=1))
        big = ctx.enter_context(tc.tile_pool(name="big", bufs=1))
        lwork = ctx.enter_context(tc.tile_pool(name="lwork", bufs=2))
        ework = ctx.enter_context(tc.tile_pool(name="ework", bufs=3))
        uwork = ctx.enter_context(tc.tile_pool(name="uwork", bufs=4))
        fwork = ctx.enter_context(tc.tile_pool(name="fwork", bufs=1))
        outp = ctx.enter_context(tc.tile_pool(name="outp", bufs=2))
        qkv_ps = ctx.enter_context(tc.tile_pool(name="qkv_ps", bufs=2, space="PSUM"))
        acc_ps = ctx.enter_context(tc.tile_pool(name="acc_ps", bufs=1, space="PSUM"))

        ident = _T(consts, [P, P], BF16, "ident")
        make_identity(nc, ident)
        ident_f = _T(consts, [P, P], FP32, "ident_f")
        make_identity(nc, ident_f)
        relh_sb = _T(consts, [P, 3], FP32, "relh")
        nc.scalar.dma_start(out=relh_sb, in_=relh[:, :])
        relw_sb = _T(consts, [P, 3], FP32, "relw")
        nc.scalar.dma_start(out=relw_sb, in_=relw[:, :])

        # ---- weights: wT[name][:, cit, cot*128:...] = w[cot-blk, cit-blk]^T
        wT = {}
        for name, wdram in (("q", wq), ("k", wk), ("v", wv)):
            wT[name] = _T(wpool, [P, 2, C], F32R, f"wT_{name}")
            for cot in range(2):
                wrow = _T(wpool, [P, C], FP32, "wrow")
                nc.scalar.dma_start(out=wrow, in_=wdram[cot * P:(cot + 1) * P, :])
                for cit in range(2):
                    ps = _T(qkv_ps, [P, P], FP32, "qkv_ps_t")
                    nc.tensor.transpose(
                        ps, in_=wrow[:, cit * P:(cit + 1) * P], identity=ident_f)
                    nc.scalar.copy(
                        out=wT[name][:, cit, cot * P:(cot + 1) * P], in_=ps)

        # (reps>1 repeats the whole load+compute for hardware timing)
        for _rep in range(reps):
            _build_body(nc, x, y, relh_sb, relw_sb, wT, ident,
                        inpool, ldp, big, lwork, ework, uwork, fwork, outp,
                        qkv_ps, acc_ps, out)
    return nc


def _build_body(nc, x, y, relh_sb, relw_sb, wT, ident,
                inpool, ldp, big, lwork, ework, uwork, fwork, outp,
                qkv_ps, acc_ps, out):
        ADD = mybir.AluOpType.add
        MULT = mybir.AluOpType.mult
        EXP = mybir.ActivationFunctionType.Exp

        # ---- inputs: DMA fp32 chunks, round to f32r (matmul operand dtype).
        # x first (k/v matmuls gate the attention pipeline), chunks
        # alternating between the two HWDGE queues (SP / Activation).
        x_r = [_T(inpool, [P, H, W], F32R, f"xr{cit}") for cit in range(2)]
        y_r = [_T(inpool, [P, H, W], F32R, f"yr{cit}") for cit in range(2)]
        qi = 0
        for dram, dsts in ((x, x_r), (y, y_r)):
            for r0 in range(0, H, 16):
                for cit in range(2):
                    sc = _T(ldp, [P, 16, W], FP32, "ld_scratch")
                    eng = nc.sync if qi % 2 == 0 else nc.scalar
                    eng.dma_start(
                        out=sc, in_=dram[cit * P:(cit + 1) * P, r0:r0 + 16, :])
                    # rounding copies: first band on the (startup-idle) DVE
                    # so band-0 matmuls start ASAP, the rest on ScalarE
                    qi += 1
                    nc.vector.tensor_copy(out=dsts[cit][:, r0:r0 + 16, :], in_=sc)

        for cot in range(2):
            rel_sb = relh_sb if cot == 0 else relw_sb

            q_sb = _T(big, [P, H, W], FP32, "q_sb")
            kpad = _T(big, [P, HP, WP], FP32, "kpad")
            vpad = _T(big, [P, HP, WP], BF16, "vpad")
            vpad_o = _T(big, [P, HP, WP], BF16, "vpad_o")
            # zero only the padding borders (interior is fully overwritten
            # by the QKV evacuations): top+bottom rows, then left+right cols
            for t in (kpad, vpad, vpad_o):
                nc.gpsimd.memset(t[:, 0:HP:HP - 1, :], 0.0)
                nc.gpsimd.memset(t[:, 1:HP - 1, 0:WP:WP - 1], 0.0)

            # ---- QKV 1x1 convs (f32r matmuls, contraction over Cin).
            # 16-row psum tiles (2 banks); each matmul targets one bank,
            # evacuations amortize the ScalarE per-instruction overhead.
            for b in range(4):  # bands of 16 rows = 1024 sites
                r = b * 16
                for wname in ("q", "k", "v"):
                    src = y_r if wname == "q" else x_r
                    ps = _T(qkv_ps, [P, 16, W], FP32, "qkv_ps_t")
                    for hb in range(2):
                        for cit in range(2):
                            nc.tensor.matmul(
                                ps[:, hb * 8:hb * 8 + 8, :],
                                lhsT=wT[wname][:, cit, cot * P:(cot + 1) * P],
                                rhs=src[cit][:, r + hb * 8:r + hb * 8 + 8, :],
                                start=(cit == 0),
                                stop=(cit == 1),
                            )
                    if wname == "q":
                        # q stays fp32 (logit path precision)
                        nc.scalar.copy(out=q_sb[:, r:r + 16, :], in_=ps)
                    elif wname == "k":
                        nc.scalar.copy(
                            out=kpad[:, 1 + r:17 + r, 1:1 + W], in_=ps)
                    else:
                        # v in bf16, plus a one-element-left-shifted copy so
                        # dj=1 window reads stay 4B-aligned (DVE 2x mode)
                        nc.scalar.copy(
                            out=vpad[:, 1 + r:17 + r, 1:1 + W], in_=ps)
                        nc.scalar.copy(
                            out=vpad_o[:, 1 + r:17 + r, 0:W], in_=ps)

            # ---- attention, chunks of RCHUNK rows
            # positions grouped 3-at-a-time sharing one rel scalar, so exp
            # runs as one ScalarE pass per group:
            #   cot0: rel depends on di -> group g = di, member m = dj
            #   cot1: rel depends on dj -> group g = dj, member m = di
            for r0 in range(0, H, RCHUNK):
                denom_ps = _T(acc_ps, [P, RCHUNK, W], FP32, "denom_ps")
                numer_ps = _T(acc_ps, [P, RCHUNK, W], FP32, "numer_ps")
                qv = q_sb[:, r0:r0 + RCHUNK, :]
                for g in range(3):
                    scal = rel_sb[:, g:g + 1]
                    l3 = _T(lwork, [P, 3, RCHUNK, W], FP32, "l3")
                    for m in range(3):
                        di, dj = (g, m) if cot == 0 else (m, g)
                        ksh = kpad[:, r0 + di:r0 + di + RCHUNK, dj:dj + W]
                        nc.vector.scalar_tensor_tensor(
                            out=l3[:, m, :, :], in0=ksh, scalar=scal, in1=qv,
                            op0=ADD, op1=MULT)
                    e3 = _T(ework, [P, 3, RCHUNK, W], BF16, "e3")
                    nc.scalar.activation(out=e3, in_=l3, func=EXP)
                    for m in range(3):
                        di, dj = (g, m) if cot == 0 else (m, g)
                        if dj == 1:
                            vsh = vpad_o[:, r0 + di:r0 + di + RCHUNK, 0:W]
                        else:
                            vsh = vpad[:, r0 + di:r0 + di + RCHUNK, dj:dj + W]
                        e_t = e3[:, m, :, :]
                        u_t = _T(uwork, [P, RCHUNK, W], BF16, "u_t")
                        nc.vector.tensor_mul(out=u_t, in0=e_t, in1=vsh)

                        first, last = (g == 0 and m == 0), (g == 2 and m == 2)
                        for hb in range(2):  # one matmul per psum bank
                            sl = slice(hb * 8, hb * 8 + 8)
                            nc.tensor.matmul(
                                denom_ps[:, sl, :], lhsT=ident, rhs=e_t[:, sl, :],
                                start=first, stop=last)
                            nc.tensor.matmul(
                                numer_ps[:, sl, :], lhsT=ident, rhs=u_t[:, sl, :],
                                start=first, stop=last)

                # 1/denom as exp(-ln(denom)) on ScalarE (same activation
                # table set as Exp; custom-DVE reciprocal doesn't compile
                # on this toolchain)
                lnd = _T(fwork, [P, RCHUNK, W], FP32, "lnd")
                nc.scalar.activation(out=lnd, in_=denom_ps,
                                     func=mybir.ActivationFunctionType.Ln)
                rec = _T(fwork, [P, RCHUNK, W], FP32, "rec")
                nc.scalar.activation(out=rec, in_=lnd, func=EXP, scale=-1.0)
                o_t = _T(outp, [P, RCHUNK, W], FP32, "o_t")
                nc.vector.tensor_mul(out=o_t, in0=numer_ps, in1=rec)
                nc.sync.dma_start(
                    out=out[cot * P:(cot + 1) * P, r0:r0 + RCHUNK, :], in_=o_t)


# ------------------------------------------------------------ entry points
def make_in_maps(x, y, wq, wk, wv, rel_h, rel_w):
    relh = np.ascontiguousarray(rel_h[:, 0, 0, :, 0], dtype=np.float32)  # [128,3]
    relw = np.ascontiguousarray(rel_w[:, 0, 0, 0, :], dtype=np.float32)  # [128,3]
    shared = {
        "wq": np.ascontiguousarray(wq, np.float32),
        "wk": np.ascontiguousarray(wk, np.float32),
        "wv": np.ascontiguousarray(wv, np.float32),
        "relh": relh,
        "relw": relw,
    }
    maps = []
    for i in range(N_CORES):
        maps.append({
            "x": np.ascontiguousarray(x[i], np.float32),
            "y": np.ascontiguousarray(y[i], np.float32),
            **shared,
        })
    return maps


_CACHED_NC = None


def kernel(x, y, wq, wk, wv, rel_h, rel_w):
    global _CACHED_NC
    _patch_compiler()
    from concourse.bass_utils import run_bass_kernel_spmd

    if _CACHED_NC is None:
        _CACHED_NC = build_nc()
    nc = _CACHED_NC
    in_maps = make_in_maps(x, y, wq, wk, wv, rel_h, rel_w)
    res = run_bass_kernel_spmd(nc, in_maps, core_ids=list(range(N_CORES)))
    out = np.stack([res.results[i]["out"] for i in range(N_CORES)], axis=0)
    return out.astype(np.float32)


# revision 27
# speedup vs baseline: 1.0637x; 1.0214x over previous
"""Self-contained Trainium2 Bass kernel: per-channel 3x3-window attention
(nn_AttentionConv).  Runs SPMD on 8 NeuronCores, data-parallel over batch
(B=8 -> one batch element per core, no collectives).

Math per (b, c, h, w):
  q = wq @ y;  k = wk @ pad(x);  v = wv @ pad(x)          (1x1 convs)
  logit[t] = q * (k_win[t] + rel[t]),  t over the 3x3 window
    rel[t] = rel_h[c, di] for c < 128, rel_w[c-128, dj] otherwise
  out = sum_t softmax_t(logit) * v_win[t]

Engine plan (per core):
  TensorE : f32r QKV matmuls; identity-matmul PSUM accumulation of the
            softmax numerator/denominator (absorbs all elementwise adds)
  VectorE : fused (k_shift + rel)*q logits (fp32, scalar_tensor_tensor),
            bf16 e*v_shift products at 2x, final numer*recip(denom)
  ScalarE : exp (fp32 in -> bf16 out), PSUM evacuations
  Softmax runs without max-subtraction: |logit| < ~45 for this input
  scale, exp stays inside fp32/bf16 range.
"""

import json
from contextlib import ExitStack

import numpy as np

import concourse.bass as bass
import concourse.tile as tile
from concourse import mybir
from concourse.masks import make_identity

# ---------------------------------------------------------------- constants
P = 128          # SBUF partitions
C = 256          # channels in/out
H = W = 64
HP = WP = 66     # padded spatial
RCHUNK = 16      # rows per attention chunk (psum: 2 banks per accumulator)
POS = [(di, dj) for di in range(3) for dj in range(3)]
FP32 = mybir.dt.float32
F32R = mybir.dt.float32r
BF16 = mybir.dt.bfloat16
N_CORES = 8

# --------------------------------------------------------------- BIR fixup
# This container's walrus build accepts at most ONE sync wait per
# instruction; Tile can emit more.  Split extras onto same-engine NoOps
# inserted immediately before the instruction.


def _fix_bir_waits(bir_json: bytes) -> bytes:
    j = json.loads(bir_json)
    n = 0
    for f in j.get("functions", []):
        for b in f.get("blocks", []):
            out = []
            for inst in b.get("instructions", []):
                si = inst.get("sync_info")
                waits = (si or {}).get("on_wait") or []
                if len(waits) > 1:
                    for w in waits[:-1]:
                        n += 1
                        out.append({
                            "debug": inst.get("debug", 0),
                            "engine": inst["engine"],
                            "ins": [],
                            "outs": [],
                            "name": f"WFIX-{n}",
                            "opcode": "NoOp",
                            "sync_info": {"on_update": [], "on_wait": [w]},
                        })
                    si["on_wait"] = [waits[-1]]
                out.append(inst)
            b["instructions"] = out
    return json.dumps(j).encode()


_PATCHED = False


def _patch_compiler():
    global _PATCHED
    if _PATCHED:
        return
    import concourse.bass2jax as bass2jax
    import concourse.bass_utils as bass_utils

    orig = bass_utils.compile_bir_kernel

    def patched(bir_json, tmpdir, neff_name="file.neff"):
        if isinstance(bir_json, str):
            bir_json = bir_json.encode()
        return orig(_fix_bir_waits(bir_json), tmpdir, neff_name)

    bass_utils.compile_bir_kernel = patched
    bass2jax.compile_bir_kernel = patched
    _PATCHED = True


def _T(pool, shape, dtype, nm):
    return pool.tile(shape, dtype, name=nm, tag=nm)


# ------------------------------------------------------------ kernel build
def build_nc(reps: int = 1) -> bass.Bass:
    nc = bass.Bass()
    x = nc.declare_dram_parameter("x", [C, H, W], FP32, isOutput=False)
    y = nc.declare_dram_parameter("y", [C, H, W], FP32, isOutput=False)
    wq = nc.declare_dram_parameter("wq", [C, C], FP32, isOutput=False)
    wk = nc.declare_dram_parameter("wk", [C, C], FP32, isOutput=False)
    wv = nc.declare_dram_parameter("wv", [C, C], FP32, isOutput=False)
    relh = nc.declare_dram_parameter("relh", [P, 3], FP32, isOutput=False)
    relw = nc.declare_dram_parameter("relw", [P, 3], FP32, isOutput=False)
    out = nc.declare_dram_parameter("out", [C, H, W], FP32, isOutput=True)

    ADD = mybir.AluOpType.add
    MULT = mybir.AluOpType.mult
    EXP = mybir.ActivationFunctionType.Exp

    with tile.TileContext(nc) as tc, ExitStack() as ctx:
        consts = ctx.enter_context(tc.tile_pool(name="consts", bufs=1))
        inpool = ctx.enter_context(tc.tile_pool(name="inpool", bufs=1))
        ldp = ctx.enter_context(tc.tile_pool(name="ldp", bufs=4))
        wpool = ctx.enter_context(tc.tile_pool(name="wpool", bufs=1))
        big = ctx.enter_context(tc.tile_pool(name="big", bufs=1))
        lwork = ctx.enter_context(tc.tile_pool(name="lwork", bufs=2))
        ework = ctx.enter_context(tc.tile_pool(name="ework", bufs=3))
        uwork = ctx.enter_context(tc.tile_pool(name="uwork", bufs=4))
        fwork = ctx.enter_context(tc.tile_pool(name="fwork", bufs=1))
        outp = ctx.enter_context(tc.tile_pool(name="outp", bufs=2))
        qkv_ps = ctx.enter_context(tc.tile_pool(name="qkv_ps", bufs=2, space="PSUM"))
        acc_ps = ctx.enter_context(tc.tile_pool(name="acc_ps", bufs=1, space="PSUM"))

        ident = _T(consts, [P, P], BF16, "ident")
        make_identity(nc, ident)
        ident_f = _T(consts, [P, P], FP32, "ident_f")
        make_identity(nc, ident_f)
        relh_sb = _T(consts, [P, 3], FP32, "relh")
        nc.sync.dma_start(out=relh_sb, in_=relh[:, :])
        relw_sb = _T(consts, [P, 3], FP32, "relw")
        nc.sync.dma_start(out=relw_sb, in_=relw[:, :])

        # ---- weights: wT[name][:, cit, cot*128:...] = w[cot-blk, cit-blk]^T
        wT = {}
        for name, wdram in (("q", wq), ("k", wk), ("v", wv)):
            wT[name] = _T(wpool, [P, 2, C], F32R, f"wT_{name}")
            for cot in range(2):
                wrow = _T(wpool, [P, C], FP32, "wrow")
                nc.sync.dma_start(out=wrow, in_=wdram[cot * P:(cot + 1) * P, :])
                for cit in range(2):
                    ps = _T(qkv_ps, [P, P], FP32, "qkv_ps_t")
                    nc.tensor.transpose(
                        ps, in_=wrow[:, cit * P:(cit + 1) * P], identity=ident_f)
                    nc.scalar.copy(
                        out=wT[name][:, cit, cot * P:(cot + 1) * P], in_=ps)

        # (reps>1 repeats the whole load+compute for hardware timing)
        for _rep in range(reps):
            _build_body(nc, x, y, relh_sb, relw_sb, wT, ident,
                        inpool, ldp, big, lwork, ework, uwork, fwork, outp,
                        qkv_ps, acc_ps, out)
    return nc


def _build_body(nc, x, y, relh_sb, relw_sb, wT, ident,
                inpool, ldp, big, lwork, ework, uwork, fwork, outp,
                qkv_ps, acc_ps, out):
        ADD = mybir.AluOpType.add
        MULT = mybir.AluOpType.mult
        EXP = mybir.ActivationFunctionType.Exp

        # ---- inputs: DMA fp32 chunks, round to f32r (matmul operand dtype).
        # x first (k/v matmuls gate the attention pipeline), chunks
        # alternating between the two HWDGE queues (SP / Activation).
        x_r = [_T(inpool, [P, H, W], F32R, f"xr{cit}") for cit in range(2)]
        y_r = [_T(inpool, [P, H, W], F32R, f"yr{cit}") for cit in range(2)]
        qi = 0
        for dram, dsts in ((x, x_r), (y, y_r)):
            for r0 in range(0, H, 16):
                for cit in range(2):
                    sc = _T(ldp, [P, 16, W], FP32, "ld_scratch")
                    eng = nc.sync if qi % 2 == 0 else nc.scalar
                    eng.dma_start(
                        out=sc, in_=dram[cit * P:(cit + 1) * P, r0:r0 + 16, :])
                    # rounding copies: first band on the (startup-idle) DVE
                    # so band-0 matmuls start ASAP, the rest on ScalarE
                    qi += 1
                    nc.vector.tensor_copy(out=dsts[cit][:, r0:r0 + 16, :], in_=sc)

        for cot in range(2):
            rel_sb = relh_sb if cot == 0 else relw_sb

            q_sb = _T(big, [P, H, W], FP32, "q_sb")
            kpad = _T(big, [P, HP, WP], FP32, "kpad")
            vpad = _T(big, [P, HP, WP], BF16, "vpad")
            vpad_o = _T(big, [P, HP, WP], BF16, "vpad_o")
            # zero only the padding borders (interior is fully overwritten
            # by the QKV evacuations): top+bottom rows, then left+right cols
            for t in (kpad, vpad, vpad_o):
                nc.vector.memset(t[:, 0:HP:HP - 1, :], 0.0)
                nc.vector.memset(t[:, 1:HP - 1, 0:WP:WP - 1], 0.0)

            # ---- QKV 1x1 convs (f32r matmuls, contraction over Cin).
            # 16-row psum tiles (2 banks); each matmul targets one bank,
            # evacuations amortize the ScalarE per-instruction overhead.
            for b in range(4):  # bands of 16 rows = 1024 sites
                r = b * 16
                for wname in ("q", "k", "v"):
                    src = y_r if wname == "q" else x_r
                    ps = _T(qkv_ps, [P, 16, W], FP32, "qkv_ps_t")
                    for hb in range(2):
                        for cit in range(2):
                            nc.tensor.matmul(
                                ps[:, hb * 8:hb * 8 + 8, :],
                                lhsT=wT[wname][:, cit, cot * P:(cot + 1) * P],
                                rhs=src[cit][:, r + hb * 8:r + hb * 8 + 8, :],
                                start=(cit == 0),
                                stop=(cit == 1),
                            )
                    if wname == "q":
                        # q stays fp32 (logit path precision)
                        nc.scalar.copy(out=q_sb[:, r:r + 16, :], in_=ps)
                    elif wname == "k":
                        nc.scalar.copy(
                            out=kpad[:, 1 + r:17 + r, 1:1 + W], in_=ps)
                    else:
                        # v in bf16, plus a one-element-left-shifted copy so
                        # dj=1 window reads stay 4B-aligned (DVE 2x mode)
                        nc.scalar.copy(
                            out=vpad[:, 1 + r:17 + r, 1:1 + W], in_=ps)
                        nc.scalar.copy(
                            out=vpad_o[:, 1 + r:17 + r, 0:W], in_=ps)

            # ---- attention, chunks of RCHUNK rows
            # positions grouped 3-at-a-time sharing one rel scalar, so exp
            # runs as one ScalarE pass per group:
            #   cot0: rel depends on di -> group g = di, member m = dj
            #   cot1: rel depends on dj -> group g = dj, member m = di
            for r0 in range(0, H, RCHUNK):
                denom_ps = _T(acc_ps, [P, RCHUNK, W], FP32, "denom_ps")
                numer_ps = _T(acc_ps, [P, RCHUNK, W], FP32, "numer_ps")
                qv = q_sb[:, r0:r0 + RCHUNK, :]
                for g in range(3):
                    scal = rel_sb[:, g:g + 1]
                    l3 = _T(lwork, [P, 3, RCHUNK, W], FP32, "l3")
                    for m in range(3):
                        di, dj = (g, m) if cot == 0 else (m, g)
                        ksh = kpad[:, r0 + di:r0 + di + RCHUNK, dj:dj + W]
                        nc.vector.scalar_tensor_tensor(
                            out=l3[:, m, :, :], in0=ksh, scalar=scal, in1=qv,
                            op0=ADD, op1=MULT)
                    e3 = _T(ework, [P, 3, RCHUNK, W], BF16, "e3")
                    nc.scalar.activation(out=e3, in_=l3, func=EXP)
                    for m in range(3):
                        di, dj = (g, m) if cot == 0 else (m, g)
                        if dj == 1:
                            vsh = vpad_o[:, r0 + di:r0 + di + RCHUNK, 0:W]
                        else:
                            vsh = vpad[:, r0 + di:r0 + di + RCHUNK, dj:dj + W]
                        e_t = e3[:, m, :, :]
                        u_t = _T(uwork, [P, RCHUNK, W], BF16, "u_t")
                        nc.vector.tensor_mul(out=u_t, in0=e_t, in1=vsh)

                        first, last = (g == 0 and m == 0), (g == 2 and m == 2)
                        for hb in range(2):  # one matmul per psum bank
                            sl = slice(hb * 8, hb * 8 + 8)
                            nc.tensor.matmul(
                                denom_ps[:, sl, :], lhsT=ident, rhs=e_t[:, sl, :],
                                start=first, stop=last)
                            nc.tensor.matmul(
                                numer_ps[:, sl, :], lhsT=ident, rhs=u_t[:, sl, :],
                                start=first, stop=last)

                # 1/denom as exp(-ln(denom)) on ScalarE (same activation
                # table set as Exp; custom-DVE reciprocal doesn't compile
                # on this toolchain)
                lnd = _T(fwork, [P, RCHUNK, W], FP32, "lnd")
                nc.scalar.activation(out=lnd, in_=denom_ps,
                                     func=mybir.ActivationFunctionType.Ln)
                rec = _T(fwork, [P, RCHUNK, W], FP32, "rec")
                nc.scalar.activation(out=rec, in_=lnd, func=EXP, scale=-1.0)
                o_t = _T(outp, [P, RCHUNK, W], FP32, "o_t")
                nc.vector.tensor_mul(out=o_t, in0=numer_ps, in1=rec)
                nc.sync.dma_start(
                    out=out[cot * P:(cot + 1) * P, r0:r0 + RCHUNK, :], in_=o_t)


# ------------------------------------------------------------ entry points
def make_in_maps(x, y, wq, wk, wv, rel_h, rel_w):
    relh = np.ascontiguousarray(rel_h[:, 0, 0, :, 0], dtype=np.float32)  # [128,3]
    relw = np.ascontiguousarray(rel_w[:, 0, 0, 0, :], dtype=np.float32)  # [128,3]
    shared = {
        "wq": np.ascontiguousarray(wq, np.float32),
        "wk": np.ascontiguousarray(wk, np.float32),
        "wv": np.ascontiguousarray(wv, np.float32),
        "relh": relh,
        "relw": relw,
    }
    maps = []
    for i in range(N_CORES):
        maps.append({
            "x": np.ascontiguousarray(x[i], np.float32),
            "y": np.ascontiguousarray(y[i], np.float32),
            **shared,
        })
    return maps


_CACHED_NC = None


def kernel(x, y, wq, wk, wv, rel_h, rel_w):
    global _CACHED_NC
    _patch_compiler()
    from concourse.bass_utils import run_bass_kernel_spmd

    if _CACHED_NC is None:
        _CACHED_NC = build_nc()
    nc = _CACHED_NC
    in_maps = make_in_maps(x, y, wq, wk, wv, rel_h, rel_w)
    res = run_bass_kernel_spmd(nc, in_maps, core_ids=list(range(N_CORES)))
    out = np.stack([res.results[i]["out"] for i in range(N_CORES)], axis=0)
    return out.astype(np.float32)
